# revision 33
# baseline (speedup 1.0000x reference)
"""Causal self-attention (B=2, T=2048, C=1024, H=16, D=64) on 8 NeuronCores.

Sharding: core = (batch b, head-group g); each of the 8 cores handles one
batch and 4 of the 16 heads (data parallel on B, tensor parallel on heads).
Each core computes q/k/v projections for its heads, rope, causal softmax
attention, and a partial out-projection; the host sums the 4 per-batch
partials and adds bout.

All matmul operands are bf16 (1 col/cycle PE stream vs 2 for fp32r), PSUM
accumulation fp32. Device dataflow (per core):
  - host passes x[b].T in bf16 so the contraction dim (C) lands on partitions
  - q,k are produced directly in [dim, t] layout ("A"=low rotary halves of
    4 heads stacked, "B"=high halves); bias added via DVE tensor_scalar
    (per-partition AP); rope applied with DVE/GpSimd in bf16
  - S^T = K~^T Q~ per 128k x 512q block, 4 heads packed into the PE array
    via tile_position row groups (contraction=32 each for A/B parts; the
    4 row groups stream concurrently)
  - softmax without max-subtraction (logits are O(1) for this model):
    exp on ScalarE with the 1/8 scale folded in, bf16 out; causal masking
    by multiplying diagonal blocks with constant 0/1 masks on DVE/GpSimd
  - O^T = V_aug^T expS^T accumulated over k blocks, where V_aug carries a
    ones column (folded into a K=1 bias-row matmul together with the v
    bias) so row 64 of the PSUM accumulator is the softmax denominator;
    divide via DVE reciprocal + K=1 PE broadcast of the recip, multiplied
    straight out of PSUM
  - partial out-projection [t,c] = (O^T)^T Wout_rows, bf16 out, summed on
    host in fp32
"""
import sys
sys.path.insert(0, '/opt/trn_rl_repo')

import numpy as np
import ml_dtypes
from contextlib import ExitStack

import concourse.bass as bass
import concourse.tile as tile
from concourse import mybir
from concourse.bass_utils import run_bass_kernel_spmd

B, T, C, H, D = 2, 2048, 1024, 16, 64
HPC = 4          # heads per core
G = H // HPC     # head groups (cores per batch)
N_CORES = B * G
SCALE = 1.0 / np.sqrt(D)
P = 128
QT = 512         # q tile width
TT = T // QT     # q tiles
NKB = T // P     # 128-wide k blocks
NTB = T // P     # 128-wide t blocks
NCC = C // P     # 128-deep contraction chunks
F32 = mybir.dt.float32
BF16 = mybir.dt.bfloat16
BF = ml_dtypes.bfloat16


def _tril_mask():
    p = np.arange(P)[:, None]
    f = np.arange(P)[None, :]
    return (p <= f).astype(np.float32)


# walrus in this toolchain can't encode >1 sem wait on one instruction
# ("Too many sync wait commands"); split excess waits onto preceding NoOps.
def _split_waits(nc, maxw=1):
    for f in nc.m.functions:
        for bb in f.blocks:
            out = []
            for inst in bb.instructions:
                si = getattr(inst, 'sync_info', None)
                if si is not None and si.on_wait and len(si.on_wait) > maxw:
                    waits = list(si.on_wait)
                    extra, keep = waits[:-maxw], waits[-maxw:]
                    for i in range(0, len(extra), maxw):
                        out.append(mybir.InstNoOp(
                            name=f"{inst.name}-wsplit{i}",
                            sync_info=mybir.SyncInfo(
                                on_wait=extra[i:i + maxw], on_update=[]),
                            bass_nofuse=True,
                            engine=inst.engine,
                        ))
                    inst.sync_info = mybir.SyncInfo(
                        on_wait=keep, on_update=list(si.on_update or []))
                out.append(inst)
            bb.instructions[:] = out


def build_nc(split=True):
    nc = bass.Bass()
    xT = nc.dram_tensor("xT", [C, T], BF16, kind="ExternalInput")
    # weights pre-arranged on host to [p, chunk*n] so DMA lines are contiguous
    wq = nc.dram_tensor("wq", [P, NCC * 256], BF16, kind="ExternalInput")
    wk = nc.dram_tensor("wk", [P, NCC * 256], BF16, kind="ExternalInput")
    wv = nc.dram_tensor("wv", [P, NCC * 260], BF16, kind="ExternalInput")
    bqk = nc.dram_tensor("bqk", [P, 4], F32, kind="ExternalInput")  # qA qB kA kB
    bvrow = nc.dram_tensor("bvrow", [1, 260], BF16, kind="ExternalInput")
    scs = nc.dram_tensor("scs", [P, 2 * T], BF16, kind="ExternalInput")  # sin|cos
    wout = nc.dram_tensor("wout", [P, 2 * C], BF16, kind="ExternalInput")
    y = nc.dram_tensor("y", [T, C], BF16, kind="ExternalOutput")
    masks_d = nc.inline_tensor(_tril_mask().astype(BF), name="cmasks")

    with tile.TileContext(nc) as tc:
        with ExitStack() as ctx:
            # ---- resident pools ----
            wpool = ctx.enter_context(tc.tile_pool(name="wts", bufs=1))
            qkpool = ctx.enter_context(tc.tile_pool(name="qk", bufs=1))
            vpool = ctx.enter_context(tc.tile_pool(name="v", bufs=1))
            otpool = ctx.enter_context(tc.tile_pool(name="ot", bufs=1))

            bqk_sb = wpool.tile([P, 4], F32, tag="bqk")
            bvrow_sb = wpool.tile([1, 260], BF16, tag="bvrow")
            # wq/wk split lo/hi so projections start on the first half
            wq_sb = [wpool.tile([P, NCC // 2, 256], BF16, tag=f"wq{i}",
                                name=f"wq{i}") for i in range(2)]
            wk_sb = [wpool.tile([P, NCC // 2, 256], BF16, tag=f"wk{i}",
                                name=f"wk{i}") for i in range(2)]
            wv_sb = wpool.tile([P, NCC, 260], BF16, tag="wv")
            # sin/cos per q-tile: [tt][P, 2, QT] (0=sin, 1=cos)
            scs_sb = [wpool.tile([P, 2, QT], BF16, tag=f"scs{t}",
                                 name=f"scs{t}") for t in range(TT)]
            wout_sb = wpool.tile([P, 2, C], BF16, tag="wout")
            masks_sb = wpool.tile([P, P], BF16, tag="masks")
            ones_sb = wpool.tile([P, P], BF16, tag="ones")
            nc.vector.memset(ones_sb[:], 1.0)

            # q/k in rotary-half layout: A = low halves of 4 heads, B = high
            qA = qkpool.tile([P, T], BF16, tag="qA")
            qB = qkpool.tile([P, T], BF16, tag="qB")
            kA = qkpool.tile([P, T], BF16, tag="kA")
            kB = qkpool.tile([P, T], BF16, tag="kB")
            qk_tiles = [qA, qB, kA, kB]
            w_of = {0: wq_sb, 1: wq_sb, 2: wk_sb, 3: wk_sb}
            col_of = {0: 0, 1: 128, 2: 0, 3: 128}

            def scs_dma(t):
                # host lays scs out as [P, TT, 2, QT] flattened
                nc.sync.dma_start(
                    scs_sb[t].rearrange("p a q -> p (a q)"),
                    scs[:, t * 2 * QT:(t + 1) * 2 * QT])

            # V tiles [t-block, 2, 4*65] (65th col per head is ones, via
            # bvrow); two t-blocks share a tile
            v_tiles = [vpool.tile([P, 2, 260], BF16, tag=f"v{tp}",
                                  name=f"v{tp}")
                       for tp in range(NTB // 2)]

            def vt(tb):
                return v_tiles[tb // 2][:, tb % 2, :]

            # O^T: heads 0,1 stacked / heads 2,3 stacked
            ot_sb = [otpool.tile([P, T], BF16, tag=f"otsb{i}", name=f"otsb{i}")
                     for i in range(2)]

            with ExitStack() as stream:
                xpool = stream.enter_context(tc.tile_pool(name="x", bufs=2))
                rtmp = stream.enter_context(tc.tile_pool(name="rtmp", bufs=4))
                ps_s = stream.enter_context(
                    tc.tile_pool(name="pss", bufs=2, space="PSUM"))
                ps_ot = stream.enter_context(
                    tc.tile_pool(name="psot", bufs=2, space="PSUM"))
                espool = stream.enter_context(tc.tile_pool(name="es", bufs=5))
                dpool = stream.enter_context(tc.tile_pool(name="dv", bufs=2))
                opool = stream.enter_context(tc.tile_pool(name="osb", bufs=2))

                xt = {}

                def load_xt(tt):
                    for cc in range(NCC):
                        t = xpool.tile([P, QT], BF16, tag=f"x{cc}",
                                       name=f"x{cc}_{tt}")
                        nc.sync.dma_start(
                            t[:], xT[cc * P:(cc + 1) * P,
                                     tt * QT:(tt + 1) * QT])
                        xt[(cc, tt)] = t

                # q (or k) projection for one tt: A and B parts in the two
                # banks of one PSUM slot, bias-add via DVE per-partition
                # scalar.
                def qkproj(qk, tt):
                    psa = ps_s.tile([P, 2, QT], F32, tag="pss",
                                    name="qk_a")
                    for j in range(2):
                        jb = 2 * qk + j
                        wsb, c0 = w_of[jb], col_of[jb]
                        for cc in range(NCC):
                            nc.tensor.matmul(
                                psa[:, j, :],
                                wsb[cc // 4][:, cc % 4, c0:c0 + 128],
                                xt[(cc, tt)][:],
                                start=(cc == 0), stop=(cc == NCC - 1))
                    for j in range(2):
                        jb = 2 * qk + j
                        dst = qk_tiles[jb][:, tt * QT:(tt + 1) * QT]
                        nc.vector.tensor_scalar_add(dst, psa[:, j, :],
                                                    bqk_sb[:, jb:jb + 1])

                def rope(pair, tt):
                    At, Bt = qk_tiles[2 * pair], qk_tiles[2 * pair + 1]
                    s = slice(tt * QT, (tt + 1) * QT)
                    t1 = rtmp.tile([P, QT], BF16, tag="rt", name="rt1")
                    t2 = rtmp.tile([P, QT], BF16, tag="rt", name="rt2")
                    t3 = rtmp.tile([P, QT], BF16, tag="rt", name="rt3")
                    cosr = scs_sb[tt][:, 1, :]
                    sinr = scs_sb[tt][:, 0, :]
                    nc.gpsimd.tensor_mul(t1[:], At[:, s], cosr)
                    nc.gpsimd.tensor_mul(t2[:], Bt[:, s], sinr)
                    nc.vector.tensor_mul(t3[:], At[:, s], sinr)
                    nc.vector.tensor_sub(At[:, s], t1[:], t2[:])
                    nc.vector.tensor_mul(Bt[:, s], Bt[:, s], cosr)
                    nc.vector.tensor_add(Bt[:, s], Bt[:, s], t3[:])

                def vproj(tp):
                    # two t-blocks (2*tp, 2*tp+1) into one PSUM slot
                    ps = ps_s.tile([P, 2, QT], F32, tag="pss",
                                   name="psv")
                    for u in range(2):
                        tb = 2 * tp + u
                        psu = ps[:, u, 0:260]
                        for cc in range(NCC):
                            nc.tensor.matmul(
                                psu, xt[(cc, tb // 4)][:, (tb % 4) * P:
                                                       (tb % 4 + 1) * P],
                                wv_sb[:, cc, :],
                                start=(cc == 0), stop=False)
                        # bias row + ones column: out[t, :] += 1 * bvrow
                        nc.tensor.matmul(
                            psu, ones_sb[0:1, :], bvrow_sb[:],
                            start=False, stop=True)
                    nc.vector.tensor_copy(v_tiles[tp][:], ps[:, :, 0:260])

                def divides_a_pp(ot2, pp):
                    otf2 = dpool.tile([65, 2, QT], BF16, tag="otf",
                                      name="otf")
                    nc.vector.tensor_copy(otf2[:], ot2[pp][:])
                    dn = dpool.tile([P, 8], BF16, tag="dn", name="dn")
                    nc.sync.dma_start(
                        dn[:], otf2[64:65, :, :].rearrange(
                            "a b c -> a (b c)"))
                    with nc.allow_low_precision(
                            reason="softmax denom reciprocal, bf16 ok"):
                        nc.vector.reciprocal(dn[:], dn[:])
                    rr = dpool.tile([1, 2, QT], BF16, tag="rr", name="rr")
                    nc.sync.dma_start(
                        rr[0:1, :, :].rearrange("a b c -> a (b c)"),
                        dn[:])
                    return [(otf2, rr)]

                def divides_b(tt, pend):
                    for pp in range(2):
                        otf2, rr = pend[pp]
                        rb = ps_s.tile([P, 2, QT], F32, tag="pss",
                                       name="rb")
                        for j in range(2):
                            nc.tensor.matmul(
                                rb[0:64, j, :], ones_sb[0:1, 0:64],
                                rr[0:1, j, :], start=True, stop=True)
                        for j in range(2):
                            dst = ot_sb[pp][64 * j:64 * j + 64,
                                            tt * QT:(tt + 1) * QT]
                            nc.vector.tensor_mul(dst, otf2[0:64, j, :],
                                                 rb[0:64, j, :])

                def outproj(tb):
                    o_sb = opool.tile([P, C], BF16, tag="osb", name="osb")
                    ps = ps_s.tile([P, 2, QT], F32, tag="pss",
                                   name="pso")
                    for nt in range(2):
                        for rc in range(2):
                            nc.tensor.matmul(
                                ps[:, nt, :], ot_sb[rc][:, tb * P:(tb + 1) * P],
                                wout_sb[:, rc, nt * 512:(nt + 1) * 512],
                                start=(rc == 0), stop=(rc == 1))
                    nc.vector.tensor_copy(
                        o_sb.rearrange("p (a n) -> p a n", a=2), ps)
                    nc.sync.dma_start(y[tb * P:(tb + 1) * P, :], o_sb[:])

                # ---- prologue: tile 0's inputs and projections ----
                # DMA priority: x/wq/wk/scs feed the first scores; wv feeds
                # the prologue vprojs; wout is deferred into the work queue.
                load_xt(0)
                half = NCC // 2 * 256
                for i in range(2):
                    nc.sync.dma_start(
                        wq_sb[i].rearrange("p o n -> p (o n)"),
                        wq[:, i * half:(i + 1) * half])
                    nc.sync.dma_start(
                        wk_sb[i].rearrange("p o n -> p (o n)"),
                        wk[:, i * half:(i + 1) * half])
                scs_dma(0)
                nc.sync.dma_start(masks_sb[:], masks_d[:])
                nc.sync.dma_start(bqk_sb[:], bqk[:])
                nc.sync.dma_start(bvrow_sb[:], bvrow[:])
                nc.sync.dma_start(
                    wv_sb.rearrange("p o n -> p (o n)"), wv[:])
                qkproj(0, 0)
                qkproj(1, 0)
                rope(0, 0)
                rope(1, 0)
                vproj(0)
                vproj(1)

                # ---- streaming attention with injected work ----
                queue = []  # closures of next-tile + prev-tile work
                prev = None
                for tt in range(TT):
                    nk = 4 * tt + 4
                    if tt + 1 < TT:
                        ntt = tt + 1
                        load_xt(ntt)
                        if tt == 0:
                            queue += [lambda: nc.sync.dma_start(
                                wout_sb.rearrange("p a n -> p (a n)"),
                                wout[:])]
                        queue += [lambda t=ntt: scs_dma(t),
                                  lambda t=ntt: qkproj(0, t),
                                  lambda t=ntt: rope(0, t),
                                  lambda t=ntt: qkproj(1, t),
                                  lambda t=ntt: rope(1, t)]
                        queue += [lambda tp=tp: vproj(tp)
                                  for tp in range(2 * ntt, 2 * ntt + 2)]
                    ot2 = [ps_ot.tile([65, 2, QT], F32, tag="psot",
                                      name=f"psot{pp}") for pp in range(2)]

                    def scores(kblk):
                        # all 8 score MMs contiguous: 4 row groups stream
                        # concurrently (A parts x4, then B parts x4)
                        off = max(0, (kblk - 4 * tt)) * P
                        ks = slice(kblk * P, (kblk + 1) * P)
                        qs = slice(tt * QT + off, (tt + 1) * QT)
                        s2p = [ps_s.tile([P, 2, QT], F32, tag="pss",
                                         name="pss") for _ in range(2)]
                        for h in range(4):
                            hp = slice(32 * h, 32 * h + 32)
                            nc.tensor.matmul(
                                s2p[h // 2][:, h % 2, off:],
                                kA[hp, ks], qA[hp, qs],
                                start=True, stop=False,
                                tile_position=(32 * h, 0))
                        for h in range(4):
                            hp = slice(32 * h, 32 * h + 32)
                            nc.tensor.matmul(
                                s2p[h // 2][:, h % 2, off:],
                                kB[hp, ks], qB[hp, qs],
                                start=False, stop=True,
                                tile_position=(32 * h, 0))
                        return s2p, off

                    es_prev = None
                    off_prev = 0
                    pend_s = scores(0)
                    for kblk in range(nk):
                        s2p, off = pend_s
                        es2 = espool.tile([P, 4, QT], BF16, tag="es",
                                          name="es")
                        for pp in range(2):
                            nc.scalar.activation(
                                es2[:, 2 * pp:2 * pp + 2, off:],
                                s2p[pp][:, :, off:],
                                mybir.ActivationFunctionType.Exp, scale=SCALE)
                        if kblk >= 4 * tt:
                            for pp in range(2):
                                eng = nc.vector if pp == 0 else nc.gpsimd
                                eng.tensor_mul(
                                    es2[:, 2 * pp:2 * pp + 2, off:off + P],
                                    es2[:, 2 * pp:2 * pp + 2, off:off + P],
                                    masks_sb[:, None, :].to_broadcast(
                                        (P, 2, P)))
                        # PE segment: attV for the previous block first (it
                        # is ready), then next block's scores (they gate the
                        # next exp), then bulk prep work
                        if kblk > 0:
                            for h in range(4):
                                nc.tensor.matmul(
                                    ot2[h // 2][:, h % 2, off_prev:],
                                    vt(kblk - 1)[:, 65 * h:65 * h + 65],
                                    es_prev[:, h, off_prev:],
                                    start=(kblk == 1), stop=False)
                        es_prev, off_prev = es2, off
                        if kblk + 1 < nk:
                            pend_s = scores(kblk + 1)
                        # injected pipeline work; the last tile's own back
                        # half is empty, so spread the prior tile's
                        # outprojs deep into it to keep the PE warm
                        if prev is not None:
                            ptt, pend = prev
                            sched = ([7, 9, 11, 13] if tt == TT - 1
                                     else [3, 4, 5, 6])
                            if kblk == 2:
                                divides_b(ptt, pend)
                            elif kblk in sched:
                                outproj(4 * ptt + sched.index(kblk))
                                if kblk == sched[-1]:
                                    prev = None
                        rounds_left = nk - 1 - kblk
                        if queue:
                            npop = max(1, -(-len(queue) // max(1, rounds_left)))                                 if rounds_left > 0 else len(queue)
                            for _ in range(min(npop, len(queue))):
                                queue.pop(0)()
                    # final attV; interleave the per-pp divide heads so the
                    # tail chain starts as soon as each pp's O^T completes
                    pend = []
                    for pp in range(2):
                        for j in range(2):
                            h = 2 * pp + j
                            nc.tensor.matmul(
                                ot2[pp][:, j, off_prev:],
                                vt(nk - 1)[:, 65 * h:65 * h + 65],
                                es_prev[:, h, off_prev:],
                                start=(nk == 1), stop=True)
                        pend += divides_a_pp(ot2, pp)
                    prev = (tt, pend)
                # tail
                ptt, pend = prev
                divides_b(ptt, pend)
                for tb in range(4 * ptt, 4 * ptt + 4):
                    outproj(tb)

    if split:
        _split_waits(nc)
    return nc


def make_in_maps(x, rope_cache, Wqkv, bqkv, Wout, bout):
    """Host-side shard prep. Returns list of 8 in_maps (core = 4*b + g)."""
    x = np.asarray(x, np.float32)
    rope_cache = np.asarray(rope_cache, np.float32)
    Wqkv = np.asarray(Wqkv, np.float32)
    bqkv = np.asarray(bqkv, np.float32)
    Wout = np.asarray(Wout, np.float32)

    # rotary-half permutation within a head: [evens, odds]
    perm = np.concatenate([np.arange(0, D, 2), np.arange(1, D, 2)])
    sin = np.tile(rope_cache[:, 0::2].T, (4, 1))   # [128, T]
    cos = np.tile(rope_cache[:, 1::2].T, (4, 1))
    # per-q-tile blocks [sin_t | cos_t]: [128, TT*2*QT]
    scs = np.concatenate(
        [np.concatenate([sin[:, t * QT:(t + 1) * QT],
                         cos[:, t * QT:(t + 1) * QT]], axis=1)
         for t in range(TT)], axis=1).astype(BF)

    xT = [np.ascontiguousarray(x[b].T).astype(BF) for b in range(B)]

    in_maps = []
    for core in range(N_CORES):
        b, g = divmod(core, G)
        heads = range(HPC * g, HPC * g + HPC)
        # A-block: low halves (even dims) of the 4 heads; B-block: high halves
        qcols, kcols, vcols = [], [], []
        for part in range(2):  # lo, hi
            for h in heads:
                dd = h * D + perm[part * 32:(part + 1) * 32]
                qcols.extend(0 * C + dd)
                kcols.extend(1 * C + dd)
        for h in heads:
            vcols.extend(2 * C + h * D + np.arange(D))
        qcols = np.asarray(qcols)
        kcols = np.asarray(kcols)
        vcols = np.asarray(vcols)
        def chunked(w):
            # [C, n] -> [128, NCC*n]: contiguous per-partition DMA lines
            n = w.shape[1]
            return np.ascontiguousarray(
                w.reshape(NCC, P, n).transpose(1, 0, 2).reshape(P, NCC * n))

        wq_c = chunked(Wqkv[:, qcols]).astype(BF)
        wk_c = chunked(Wqkv[:, kcols]).astype(BF)
        wv_c = np.zeros((C, 260), np.float32)
        vv = Wqkv[:, vcols]
        for h in range(HPC):
            wv_c[:, 65 * h:65 * h + 64] = vv[:, 64 * h:64 * h + 64]
        wv_c = chunked(wv_c)
        bqk_c = np.stack([bqkv[qcols[:128]], bqkv[qcols[128:]],
                          bqkv[kcols[:128]], bqkv[kcols[128:]]], axis=1)
        bv_c = bqkv[vcols]
        bvrow = np.zeros((1, 260), np.float32)
        for h in range(HPC):
            bvrow[0, 65 * h:65 * h + 64] = bv_c[64 * h:64 * h + 64]
            bvrow[0, 65 * h + 64] = 1.0
        rows = np.arange(HPC * g * D, (HPC * g + HPC) * D)
        wout_c = Wout[rows, :]  # [256, C] -> [128, 2*C] chunked
        wout_c = np.ascontiguousarray(
            wout_c.reshape(2, P, C).transpose(1, 0, 2).reshape(P, 2 * C)
        ).astype(BF)
        in_maps.append({
            "xT": xT[b], "wq": wq_c, "wk": wk_c,
            "wv": np.ascontiguousarray(wv_c).astype(BF),
            "bqk": np.ascontiguousarray(bqk_c.astype(np.float32)),
            "bvrow": np.ascontiguousarray(bvrow).astype(BF),
            "scs": scs, "wout": wout_c,
        })
    return in_maps


_NC_CACHE = None


def _get_nc():
    global _NC_CACHE
    if _NC_CACHE is None:
        _NC_CACHE = build_nc()
    return _NC_CACHE


def run(inputs, trace=False):
    nc = _get_nc()
    in_maps = make_in_maps(**inputs)
    res = run_bass_kernel_spmd(nc, in_maps, list(range(N_CORES)), trace=trace)
    bout = np.asarray(inputs["bout"], np.float32)
    out = np.zeros((B, T, C), np.float32)
    for core in range(N_CORES):
        out[core // G] += res.results[core]["y"].astype(np.float32)
    out += bout[None, None, :]
    return out, res


def kernel(**inputs):
    out, _ = run(inputs)
    return out


# revision 43
# speedup vs baseline: 1.0231x; 1.0231x over previous
"""Causal self-attention (B=2, T=2048, C=1024, H=16, D=64) on 8 NeuronCores.

Sharding: core = (batch b, head-group g); each of the 8 cores handles one
batch and 4 of the 16 heads (data parallel on B, tensor parallel on heads).
Each core computes q/k/v projections for its heads, rope, causal softmax
attention, and a partial out-projection; the host sums the 4 per-batch
partials and adds bout.

All matmul operands are bf16 (1 col/cycle PE stream vs 2 for fp32r), PSUM
accumulation fp32. Device dataflow (per core):
  - host passes x[b].T in bf16 so the contraction dim (C) lands on partitions
  - q,k are produced directly in [dim, t] layout ("A"=low rotary halves of
    4 heads stacked, "B"=high halves); bias added via DVE tensor_scalar
    (per-partition AP); rope applied with DVE/GpSimd in bf16
  - S^T = K~^T Q~ per 128k x 512q block, 4 heads packed into the PE array
    via tile_position row groups (contraction=32 each for A/B parts; the
    4 row groups stream concurrently)
  - softmax without max-subtraction (logits are O(1) for this model):
    exp on ScalarE with the 1/8 scale folded in, bf16 out; causal masking
    by multiplying diagonal blocks with constant 0/1 masks on DVE/GpSimd
  - O^T = V_aug^T expS^T accumulated over k blocks, where V_aug carries a
    ones column (folded into a K=1 bias-row matmul together with the v
    bias) so row 64 of the PSUM accumulator is the softmax denominator;
    divide via DVE reciprocal + K=1 PE broadcast of the recip, multiplied
    straight out of PSUM
  - partial out-projection [t,c] = (O^T)^T Wout_rows, bf16 out, summed on
    host in fp32
"""
import sys
sys.path.insert(0, '/opt/trn_rl_repo')

import numpy as np
import ml_dtypes
from contextlib import ExitStack

import concourse.bass as bass
import concourse.tile as tile
from concourse import mybir
from concourse.bass_utils import run_bass_kernel_spmd

B, T, C, H, D = 2, 2048, 1024, 16, 64
HPC = 4          # heads per core
G = H // HPC     # head groups (cores per batch)
N_CORES = B * G
SCALE = 1.0 / np.sqrt(D)
P = 128
QT = 512         # q tile width
TT = T // QT     # q tiles
NKB = T // P     # 128-wide k blocks
NTB = T // P     # 128-wide t blocks
NCC = C // P     # 128-deep contraction chunks
F32 = mybir.dt.float32
BF16 = mybir.dt.bfloat16
BF = ml_dtypes.bfloat16


def _tril_mask():
    p = np.arange(P)[:, None]
    f = np.arange(P)[None, :]
    return (p <= f).astype(np.float32)


# walrus in this toolchain can't encode >1 sem wait on one instruction
# ("Too many sync wait commands"); split excess waits onto preceding NoOps.
def _split_waits(nc, maxw=1):
    for f in nc.m.functions:
        for bb in f.blocks:
            out = []
            for inst in bb.instructions:
                si = getattr(inst, 'sync_info', None)
                if si is not None and si.on_wait and len(si.on_wait) > maxw:
                    waits = list(si.on_wait)
                    extra, keep = waits[:-maxw], waits[-maxw:]
                    for i in range(0, len(extra), maxw):
                        out.append(mybir.InstNoOp(
                            name=f"{inst.name}-wsplit{i}",
                            sync_info=mybir.SyncInfo(
                                on_wait=extra[i:i + maxw], on_update=[]),
                            bass_nofuse=True,
                            engine=inst.engine,
                        ))
                    inst.sync_info = mybir.SyncInfo(
                        on_wait=keep, on_update=list(si.on_update or []))
                out.append(inst)
            bb.instructions[:] = out


def build_nc(split=True):
    nc = bass.Bass()
    xT = nc.dram_tensor("xT", [C, T], BF16, kind="ExternalInput")
    # weights pre-arranged on host to [p, chunk*n] so DMA lines are contiguous
    wq = nc.dram_tensor("wq", [P, NCC * 256], BF16, kind="ExternalInput")
    wk = nc.dram_tensor("wk", [P, NCC * 256], BF16, kind="ExternalInput")
    wv = nc.dram_tensor("wv", [P, NCC * 260], BF16, kind="ExternalInput")
    bqk = nc.dram_tensor("bqk", [P, 4], F32, kind="ExternalInput")  # qA qB kA kB
    bvrow = nc.dram_tensor("bvrow", [1, 260], BF16, kind="ExternalInput")
    scs = nc.dram_tensor("scs", [P, 2 * T], BF16, kind="ExternalInput")  # sin|cos
    wout = nc.dram_tensor("wout", [P, 2 * C], BF16, kind="ExternalInput")
    y = nc.dram_tensor("y", [T, C], BF16, kind="ExternalOutput")
    masks_d = nc.inline_tensor(_tril_mask().astype(BF), name="cmasks")

    with tile.TileContext(nc) as tc:
        with ExitStack() as ctx:
            # ---- resident pools ----
            wpool = ctx.enter_context(tc.tile_pool(name="wts", bufs=1))
            qkpool = ctx.enter_context(tc.tile_pool(name="qk", bufs=1))
            vpool = ctx.enter_context(tc.tile_pool(name="v", bufs=1))
            otpool = ctx.enter_context(tc.tile_pool(name="ot", bufs=1))

            bqk_sb = wpool.tile([P, 4], F32, tag="bqk")
            bvrow_sb = wpool.tile([1, 260], BF16, tag="bvrow")
            # wq/wk split lo/hi so projections start on the first half
            wq_sb = [wpool.tile([P, NCC // 2, 256], BF16, tag=f"wq{i}",
                                name=f"wq{i}") for i in range(2)]
            wk_sb = [wpool.tile([P, NCC // 2, 256], BF16, tag=f"wk{i}",
                                name=f"wk{i}") for i in range(2)]
            wv_sb = wpool.tile([P, NCC, 260], BF16, tag="wv")
            # sin/cos per q-tile: [tt][P, 2, QT] (0=sin, 1=cos)
            scs_sb = [wpool.tile([P, 2, QT], BF16, tag=f"scs{t}",
                                 name=f"scs{t}") for t in range(TT)]
            wout_sb = wpool.tile([P, 2, C], BF16, tag="wout")
            masks_sb = wpool.tile([P, P], BF16, tag="masks")
            ones_sb = wpool.tile([P, P], BF16, tag="ones")
            nc.vector.memset(ones_sb[:], 1.0)

            # q/k in rotary-half layout: A = low halves of 4 heads, B = high
            qA = qkpool.tile([P, T], BF16, tag="qA")
            qB = qkpool.tile([P, T], BF16, tag="qB")
            kA = qkpool.tile([P, T], BF16, tag="kA")
            kB = qkpool.tile([P, T], BF16, tag="kB")
            qk_tiles = [qA, qB, kA, kB]
            w_of = {0: wq_sb, 1: wq_sb, 2: wk_sb, 3: wk_sb}
            col_of = {0: 0, 1: 128, 2: 0, 3: 128}

            def scs_dma(t):
                # host lays scs out as [P, TT, 2, QT] flattened
                nc.sync.dma_start(
                    scs_sb[t].rearrange("p a q -> p (a q)"),
                    scs[:, t * 2 * QT:(t + 1) * 2 * QT])

            # V tiles [t-block, 2, 4*65] (65th col per head is ones, via
            # bvrow); two t-blocks share a tile
            v_tiles = [vpool.tile([P, 2, 260], BF16, tag=f"v{tp}",
                                  name=f"v{tp}")
                       for tp in range(NTB // 2)]

            def vt(tb):
                return v_tiles[tb // 2][:, tb % 2, :]

            # O^T: heads 0,1 stacked / heads 2,3 stacked
            ot_sb = [otpool.tile([P, T], BF16, tag=f"otsb{i}", name=f"otsb{i}")
                     for i in range(2)]

            with ExitStack() as stream:
                xpool = stream.enter_context(tc.tile_pool(name="x", bufs=2))
                rtmp = stream.enter_context(tc.tile_pool(name="rtmp", bufs=4))
                ps_s = stream.enter_context(
                    tc.tile_pool(name="pss", bufs=2, space="PSUM"))
                ps_ot = stream.enter_context(
                    tc.tile_pool(name="psot", bufs=2, space="PSUM"))
                espool = stream.enter_context(tc.tile_pool(name="es", bufs=5))
                dpool = stream.enter_context(tc.tile_pool(name="dv", bufs=2))
                opool = stream.enter_context(tc.tile_pool(name="osb", bufs=1))

                xt = {}

                def load_xt(tt):
                    for cc in range(NCC):
                        t = xpool.tile([P, QT], BF16, tag=f"x{cc}",
                                       name=f"x{cc}_{tt}")
                        nc.sync.dma_start(
                            t[:], xT[cc * P:(cc + 1) * P,
                                     tt * QT:(tt + 1) * QT])
                        xt[(cc, tt)] = t

                # q (or k) projection for one tt: A and B parts in the two
                # banks of one PSUM slot, bias-add via DVE per-partition
                # scalar.
                def qkproj(jb, tt):
                    # one 8-chunk PSUM generation per A/B part; bias-add on
                    # ScalarE (idle during projection bursts)
                    wsb, c0 = w_of[jb], col_of[jb]
                    psa = ps_s.tile([P, 2, QT], F32, tag="pss",
                                    name="qk_a")[:, 0, :]
                    for cc in range(NCC):
                        nc.tensor.matmul(
                            psa, wsb[cc // 4][:, cc % 4, c0:c0 + 128],
                            xt[(cc, tt)][:],
                            start=(cc == 0), stop=(cc == NCC - 1))
                    dst = qk_tiles[jb][:, tt * QT:(tt + 1) * QT]
                    nc.scalar.activation(
                        dst, psa, mybir.ActivationFunctionType.Identity,
                        bias=bqk_sb[:, jb:jb + 1], scale=1.0)

                def rope(pair, tt):
                    At, Bt = qk_tiles[2 * pair], qk_tiles[2 * pair + 1]
                    s = slice(tt * QT, (tt + 1) * QT)
                    t1 = rtmp.tile([P, QT], BF16, tag="rt", name="rt1")
                    t2 = rtmp.tile([P, QT], BF16, tag="rt", name="rt2")
                    t3 = rtmp.tile([P, QT], BF16, tag="rt", name="rt3")
                    cosr = scs_sb[tt][:, 1, :]
                    sinr = scs_sb[tt][:, 0, :]
                    nc.gpsimd.tensor_mul(t1[:], At[:, s], cosr)
                    nc.gpsimd.tensor_mul(t2[:], Bt[:, s], sinr)
                    nc.vector.tensor_mul(t3[:], At[:, s], sinr)
                    nc.vector.tensor_sub(At[:, s], t1[:], t2[:])
                    nc.vector.tensor_mul(Bt[:, s], Bt[:, s], cosr)
                    nc.vector.tensor_add(Bt[:, s], Bt[:, s], t3[:])

                def vproj(tb):
                    ps = ps_s.tile([P, 2, QT], F32, tag="pss",
                                   name="psv")[:, 0, 0:260]
                    for cc in range(NCC):
                        nc.tensor.matmul(
                            ps, xt[(cc, tb // 4)][:, (tb % 4) * P:
                                                  (tb % 4 + 1) * P],
                            wv_sb[:, cc, :],
                            start=(cc == 0), stop=False)
                    # bias row + ones column: out[t, :] += 1 * bvrow
                    nc.tensor.matmul(
                        ps, ones_sb[0:1, :], bvrow_sb[:],
                        start=False, stop=True)
                    nc.vector.tensor_copy(vt(tb), ps)

                def divides_a_pp(ot2, pp):
                    otf2 = dpool.tile([65, 2, QT], BF16, tag="otf",
                                      name="otf")
                    nc.vector.tensor_copy(otf2[:], ot2[pp][:])
                    dn = dpool.tile([P, 8], BF16, tag="dn", name="dn")
                    nc.sync.dma_start(
                        dn[:], otf2[64:65, :, :].rearrange(
                            "a b c -> a (b c)"))
                    with nc.allow_low_precision(
                            reason="softmax denom reciprocal, bf16 ok"):
                        nc.vector.reciprocal(dn[:], dn[:])
                    rr = dpool.tile([1, 2, QT], BF16, tag="rr", name="rr")
                    nc.sync.dma_start(
                        rr[0:1, :, :].rearrange("a b c -> a (b c)"),
                        dn[:])
                    return [(otf2, rr)]

                def divides_b(tt, pend):
                    for pp in range(2):
                        otf2, rr = pend[pp]
                        rb = ps_s.tile([P, 2, QT], F32, tag="pss",
                                       name="rb")
                        for j in range(2):
                            nc.tensor.matmul(
                                rb[0:64, j, :], ones_sb[0:1, 0:64],
                                rr[0:1, j, :], start=True, stop=True)
                        for j in range(2):
                            dst = ot_sb[pp][64 * j:64 * j + 64,
                                            tt * QT:(tt + 1) * QT]
                            nc.vector.tensor_mul(dst, otf2[0:64, j, :],
                                                 rb[0:64, j, :])

                obuf = [opool.tile([P, C], BF16, tag=f"osb{i}",
                                   name=f"osb{i}") for i in range(2)]

                def outproj(tb, nt):
                    # half of one t-block's out-projection per call
                    o_sb = obuf[tb % 2]
                    ps = ps_s.tile([P, 2, QT], F32, tag="pss",
                                   name="pso")[:, 0, :]
                    for rc in range(2):
                        nc.tensor.matmul(
                            ps, ot_sb[rc][:, tb * P:(tb + 1) * P],
                            wout_sb[:, rc, nt * 512:(nt + 1) * 512],
                            start=(rc == 0), stop=(rc == 1))
                    nc.vector.tensor_copy(
                        o_sb[:, nt * 512:(nt + 1) * 512], ps)
                    if nt == 1:
                        nc.sync.dma_start(y[tb * P:(tb + 1) * P, :], o_sb[:])

                # ---- prologue: tile 0's inputs and projections ----
                # DMA priority: x/wq/wk/scs feed the first scores; wv feeds
                # the prologue vprojs; wout is deferred into the work queue.
                load_xt(0)
                half = NCC // 2 * 256
                for i in range(2):
                    nc.sync.dma_start(
                        wq_sb[i].rearrange("p o n -> p (o n)"),
                        wq[:, i * half:(i + 1) * half])
                    nc.sync.dma_start(
                        wk_sb[i].rearrange("p o n -> p (o n)"),
                        wk[:, i * half:(i + 1) * half])
                scs_dma(0)
                nc.sync.dma_start(masks_sb[:], masks_d[:])
                nc.sync.dma_start(bqk_sb[:], bqk[:])
                nc.sync.dma_start(bvrow_sb[:], bvrow[:])
                nc.sync.dma_start(
                    wv_sb.rearrange("p o n -> p (o n)"), wv[:])
                for jb in range(4):
                    qkproj(jb, 0)
                rope(0, 0)
                rope(1, 0)
                for tb in range(4):
                    vproj(tb)

                # ---- streaming attention with injected work ----
                queue = []  # closures of next-tile + prev-tile work
                prev = None
                for tt in range(TT):
                    nk = 4 * tt + 4
                    if tt + 1 < TT:
                        ntt = tt + 1
                        load_xt(ntt)
                        if tt == 0:
                            queue += [lambda: nc.sync.dma_start(
                                wout_sb.rearrange("p a n -> p (a n)"),
                                wout[:])]
                        queue += [lambda t=ntt: scs_dma(t),
                                  lambda t=ntt: qkproj(0, t),
                                  lambda t=ntt: qkproj(1, t),
                                  lambda t=ntt: rope(0, t),
                                  lambda t=ntt: qkproj(2, t),
                                  lambda t=ntt: qkproj(3, t),
                                  lambda t=ntt: rope(1, t)]
                        queue += [lambda tb=tb: vproj(tb)
                                  for tb in range(4 * ntt, 4 * ntt + 4)]
                    ot2 = [ps_ot.tile([65, 2, QT], F32, tag="psot",
                                      name=f"psot{pp}") for pp in range(2)]

                    def scores(kblk):
                        # all 8 score MMs contiguous: 4 row groups stream
                        # concurrently (A parts x4, then B parts x4)
                        off = max(0, (kblk - 4 * tt)) * P
                        ks = slice(kblk * P, (kblk + 1) * P)
                        qs = slice(tt * QT + off, (tt + 1) * QT)
                        s2p = [ps_s.tile([P, 2, QT], F32, tag="pss",
                                         name="pss") for _ in range(2)]
                        for h in range(4):
                            hp = slice(32 * h, 32 * h + 32)
                            nc.tensor.matmul(
                                s2p[h // 2][:, h % 2, off:],
                                kA[hp, ks], qA[hp, qs],
                                start=True, stop=False,
                                tile_position=(32 * h, 0))
                        for h in range(4):
                            hp = slice(32 * h, 32 * h + 32)
                            nc.tensor.matmul(
                                s2p[h // 2][:, h % 2, off:],
                                kB[hp, ks], qB[hp, qs],
                                start=False, stop=True,
                                tile_position=(32 * h, 0))
                        return s2p, off

                    es_prev = None
                    off_prev = 0
                    pend_s = scores(0)
                    for kblk in range(nk):
                        s2p, off = pend_s
                        es2 = espool.tile([P, 4, QT], BF16, tag="es",
                                          name="es")
                        for pp in range(2):
                            nc.scalar.activation(
                                es2[:, 2 * pp:2 * pp + 2, off:],
                                s2p[pp][:, :, off:],
                                mybir.ActivationFunctionType.Exp, scale=SCALE)
                        if kblk >= 4 * tt:
                            nc.vector.tensor_mul(
                                es2[:, :, off:off + P],
                                es2[:, :, off:off + P],
                                masks_sb[:, None, :].to_broadcast(
                                    (P, 4, P)))
                        # PE segment: attV for the previous block first (it
                        # is ready), then next block's scores (they gate the
                        # next exp), then bulk prep work
                        if kblk > 0:
                            for h in range(4):
                                nc.tensor.matmul(
                                    ot2[h // 2][:, h % 2, off_prev:],
                                    vt(kblk - 1)[:, 65 * h:65 * h + 65],
                                    es_prev[:, h, off_prev:],
                                    start=(kblk == 1), stop=False)
                        es_prev, off_prev = es2, off
                        if kblk + 1 < nk:
                            pend_s = scores(kblk + 1)
                        # injected pipeline work; the last tile's own back
                        # half is empty, so spread the prior tile's
                        # outprojs deep into it to keep the PE warm
                        if prev is not None:
                            ptt, pend = prev
                            sched = (list(range(7, 15)) if tt == TT - 1
                                     else [3, 3, 4, 4, 5, 5, 6, 6])
                            if kblk == 2:
                                divides_b(ptt, pend)
                            elif kblk in sched:
                                i0 = sched.index(kblk)
                                n = sched.count(kblk)
                                for u in range(n):
                                    half = i0 + u
                                    outproj(4 * ptt + half // 2, half % 2)
                                if kblk == sched[-1]:
                                    prev = None
                        rounds_left = nk - 1 - kblk
                        if queue:
                            npop = max(1, -(-len(queue) // max(1, rounds_left)))                                 if rounds_left > 0 else len(queue)
                            for _ in range(min(npop, len(queue))):
                                queue.pop(0)()
                    # final attV; interleave the per-pp divide heads so the
                    # tail chain starts as soon as each pp's O^T completes
                    pend = []
                    for pp in range(2):
                        for j in range(2):
                            h = 2 * pp + j
                            nc.tensor.matmul(
                                ot2[pp][:, j, off_prev:],
                                vt(nk - 1)[:, 65 * h:65 * h + 65],
                                es_prev[:, h, off_prev:],
                                start=(nk == 1), stop=True)
                        pend += divides_a_pp(ot2, pp)
                    prev = (tt, pend)
                # tail
                ptt, pend = prev
                divides_b(ptt, pend)
                for tb in range(4 * ptt, 4 * ptt + 4):
                    outproj(tb, 0)
                    outproj(tb, 1)

    if split:
        _split_waits(nc)
    return nc


def make_in_maps(x, rope_cache, Wqkv, bqkv, Wout, bout):
    """Host-side shard prep. Returns list of 8 in_maps (core = 4*b + g)."""
    x = np.asarray(x, np.float32)
    rope_cache = np.asarray(rope_cache, np.float32)
    Wqkv = np.asarray(Wqkv, np.float32)
    bqkv = np.asarray(bqkv, np.float32)
    Wout = np.asarray(Wout, np.float32)

    # rotary-half permutation within a head: [evens, odds]
    perm = np.concatenate([np.arange(0, D, 2), np.arange(1, D, 2)])
    sin = np.tile(rope_cache[:, 0::2].T, (4, 1))   # [128, T]
    cos = np.tile(rope_cache[:, 1::2].T, (4, 1))
    # per-q-tile blocks [sin_t | cos_t]: [128, TT*2*QT]
    scs = np.concatenate(
        [np.concatenate([sin[:, t * QT:(t + 1) * QT],
                         cos[:, t * QT:(t + 1) * QT]], axis=1)
         for t in range(TT)], axis=1).astype(BF)

    xT = [np.ascontiguousarray(x[b].T).astype(BF) for b in range(B)]

    in_maps = []
    for core in range(N_CORES):
        b, g = divmod(core, G)
        heads = range(HPC * g, HPC * g + HPC)
        # A-block: low halves (even dims) of the 4 heads; B-block: high halves
        qcols, kcols, vcols = [], [], []
        for part in range(2):  # lo, hi
            for h in heads:
                dd = h * D + perm[part * 32:(part + 1) * 32]
                qcols.extend(0 * C + dd)
                kcols.extend(1 * C + dd)
        for h in heads:
            vcols.extend(2 * C + h * D + np.arange(D))
        qcols = np.asarray(qcols)
        kcols = np.asarray(kcols)
        vcols = np.asarray(vcols)
        def chunked(w):
            # [C, n] -> [128, NCC*n]: contiguous per-partition DMA lines
            n = w.shape[1]
            return np.ascontiguousarray(
                w.reshape(NCC, P, n).transpose(1, 0, 2).reshape(P, NCC * n))

        wq_c = chunked(Wqkv[:, qcols]).astype(BF)
        wk_c = chunked(Wqkv[:, kcols]).astype(BF)
        wv_c = np.zeros((C, 260), np.float32)
        vv = Wqkv[:, vcols]
        for h in range(HPC):
            wv_c[:, 65 * h:65 * h + 64] = vv[:, 64 * h:64 * h + 64]
        wv_c = chunked(wv_c)
        bqk_c = np.stack([bqkv[qcols[:128]], bqkv[qcols[128:]],
                          bqkv[kcols[:128]], bqkv[kcols[128:]]], axis=1)
        bv_c = bqkv[vcols]
        bvrow = np.zeros((1, 260), np.float32)
        for h in range(HPC):
            bvrow[0, 65 * h:65 * h + 64] = bv_c[64 * h:64 * h + 64]
            bvrow[0, 65 * h + 64] = 1.0
        rows = np.arange(HPC * g * D, (HPC * g + HPC) * D)
        wout_c = Wout[rows, :]  # [256, C] -> [128, 2*C] chunked
        wout_c = np.ascontiguousarray(
            wout_c.reshape(2, P, C).transpose(1, 0, 2).reshape(P, 2 * C)
        ).astype(BF)
        in_maps.append({
            "xT": xT[b], "wq": wq_c, "wk": wk_c,
            "wv": np.ascontiguousarray(wv_c).astype(BF),
            "bqk": np.ascontiguousarray(bqk_c.astype(np.float32)),
            "bvrow": np.ascontiguousarray(bvrow).astype(BF),
            "scs": scs, "wout": wout_c,
        })
    return in_maps


_NC_CACHE = None


def _get_nc():
    global _NC_CACHE
    if _NC_CACHE is None:
        _NC_CACHE = build_nc()
    return _NC_CACHE


def run(inputs, trace=False):
    nc = _get_nc()
    in_maps = make_in_maps(**inputs)
    res = run_bass_kernel_spmd(nc, in_maps, list(range(N_CORES)), trace=trace)
    bout = np.asarray(inputs["bout"], np.float32)
    out = np.zeros((B, T, C), np.float32)
    for core in range(N_CORES):
        out[core // G] += res.results[core]["y"].astype(np.float32)
    out += bout[None, None, :]
    return out, res


def kernel(**inputs):
    out, _ = run(inputs)
    return out
